# revision 14
# baseline (speedup 1.0000x reference)
"""Attention pooling kernel for Trainium2 (Bass/Tile), SPMD over 8 NeuronCores.

Reference computation (per batch b):
    scores[t] = x[b,t,:] @ q / sqrt(D) + (1-mask[b,t]) * (-1e9)
    attn      = softmax(scores)            # over t
    out[b,:]  = sum_t attn[t] * x[b,t,:]

Strategy: data-parallel over batch (4 batches per core). One pass over x
(read once from HBM, ~64 MiB/core fp32 -> the kernel is HBM-bound at
~358 GB/s/core, floor ~187 us). To hide ALL compute under the DMA stream,
everything downstream of the load runs in bf16:

  - x[b] viewed as [128 partitions, 64 cols, 512] with t = p*64 + n,
    streamed in [128, CHUNK, 512] chunks, cast fp32->bf16 in the DMA
    (SWDGE). 32 KiB contiguous per partition per chunk at CHUNK=16.
  - scores: DVE scalar_tensor_tensor (x * q_scaled, accum over d). The
    two-tensor STT flavor has no accelerated uops (1x: ~(512+151)/0.96
    = 691 ns per [128,512] tile) - this engine is the kernel's pole at
    ~94% busy, everything else hides under it.
  - exp on ScalarE in groups of G tiles; masking is a bf16 multiply by
    m (in {0,1}) AFTER exp (exact - replaces the -1e9 bias add). Scores
    are O(0.1) so no max-subtraction is needed.
  - pooled accumulation on PE (bf16): psum[1,512] += w_col.T @ x_tile,
    over all 64 tiles of the batch, grouped with the exp stage so the PE
    gets work every ~2 us.
  - Z = sum(w) via column reduce + ones-matmul. out = acc / Z (fp32).

Measured on HW: 217 us (vs 323 us fp32r baseline). A GpSimd/ScalarE
score offload and a per-column exp-with-bias variant were both tried and
measured SLOWER (238 us): the Pool TT is ~1.4 us/tile and the offloaded
columns stall the in-order PE matmul chain; keep the DVE-only stream.
"""

import os

import numpy as np

import bass_rust as _br
import concourse.bass as bass
import concourse.tile as tile
from concourse import mybir
from concourse.bass_utils import run_bass_kernel_spmd

B, T, D = 32, 8192, 512
N_CORES = 8
BC = B // N_CORES  # batches per core
P = 128  # SBUF partitions
NCOL = T // P  # 64 tiles (columns) per batch
CHUNK = int(os.environ.get("AP_CHUNK", "16"))  # tiles per DMA chunk
NCHUNK = NCOL // CHUNK
G = int(os.environ.get("AP_G", "4"))  # tiles per exp/matmul group
SCALE = 1.0 / float(np.sqrt(np.float32(D)))
XBUFS = int(os.environ.get("AP_XBUFS", "4"))
# Of every SPLITMOD score tiles, NSPLIT take the [DVE 2x-mode multiply +
# ScalarE accumulate-copy] route; the rest use the fused (1x) DVE STT.
# Balances the DVE (the pole) against ScalarE slack. Measured on HW:
# 0/3 -> 217 us, 1/3 -> 214 us, 2/5 (+XBUFS=4) -> 188 us, 2/3 -> 235 us.
NSPLIT = int(os.environ.get("AP_NSPLIT", "2"))
SPLITMOD = int(os.environ.get("AP_SPLITMOD", "5"))
GROUPROUTE = int(os.environ.get("AP_GROUPROUTE", "1"))

F32 = mybir.dt.float32
BF16 = mybir.dt.bfloat16
I32 = mybir.dt.int32


def _split_multi_waits(nc):
    """The walrus build in this container accepts only one sync-wait command
    per instruction; hoist extra waits onto standalone EventSemaphore
    instructions placed just before (same engine, program order preserved)."""
    for f in nc.m.functions:
        for b in f.blocks:
            insts = b.instructions
            new = []
            changed = False
            for inst in insts:
                si = inst.sync_info
                if si is not None and len(si.on_wait) > 1:
                    waits = list(si.on_wait)
                    for w in waits[:-1]:
                        ies = mybir.InstEventSemaphore(
                            name=f"I-waitsplit-{nc.next_id()}", ins=[], outs=[]
                        )
                        ies.engine = inst.engine
                        ies.sync_info = _br.SyncInfo(on_wait=[w], on_update=[])
                        new.append(ies)
                    inst.sync_info = _br.SyncInfo(
                        on_wait=[waits[-1]], on_update=list(si.on_update)
                    )
                    changed = True
                new.append(inst)
            if changed:
                b.instructions = new


def _build_bass():
    nc = bass.Bass(
        "TRN2", target_bir_lowering=False, debug=False, num_devices=N_CORES
    )
    x = nc.dram_tensor("x", [BC, T, D], F32, kind="ExternalInput").ap()
    mask = nc.dram_tensor("mask", [BC, T], I32, kind="ExternalInput").ap()
    q = nc.dram_tensor("pool_query", [1, 1, D], F32, kind="ExternalInput").ap()
    out = nc.dram_tensor("out", [BC, D], F32, kind="ExternalOutput").ap()

    # t = p * NCOL + n  (partition-major): per-partition rows are contiguous
    # in DRAM, so a [128, CHUNK, 512] chunk is CHUNK*2 KiB contiguous per
    # partition.
    xv = x.rearrange("b (p n) d -> b p n d", p=P)
    mv = mask.rearrange("b (p n) -> b p n", p=P)

    with tile.TileContext(nc) as tc:
        with (
            tc.tile_pool(name="const", bufs=1) as const_pool,
            tc.tile_pool(name="xp", bufs=XBUFS) as xpool,
            tc.tile_pool(name="sp", bufs=4) as spool,
            tc.tile_pool(name="bp", bufs=2) as bpool,
            tc.tile_pool(name="ep", bufs=2) as epool,
            tc.tile_pool(name="pacc", bufs=2, space="PSUM") as pacc,
            tc.tile_pool(name="pz", bufs=2, space="PSUM") as pz,
        ):
            # q broadcast to all 128 partitions (one-time, 256 KB)
            q_bcast = const_pool.tile([P, D], F32)
            q_src = bass.AP(tensor=q.tensor, offset=q.offset, ap=[[0, P], [1, D]])
            nc.gpsimd.dma_start(out=q_bcast, in_=q_src)

            ones_col = const_pool.tile([P, 1], F32)
            nc.vector.memset(ones_col, 1.0)

            # pre-scaled bf16 query (folds the 1/sqrt(D) into q; done on
            # ScalarE so the DVE never enters a 2-port mode that would lock
            # GpSimd out of the SWDGE descriptor rings)
            q_bf = const_pool.tile([P, D], BF16)
            nc.scalar.activation(
                out=q_bf,
                in_=q_bcast,
                func=mybir.ActivationFunctionType.Copy,
                scale=float(SCALE),
            )

            for b in range(BC):
                # mask -> bf16 multiplier (1 valid, 0 pad), applied post-exp
                m_i32 = bpool.tile([P, NCOL], I32)
                nc.sync.dma_start(out=m_i32, in_=mv[b])
                m_f = bpool.tile([P, NCOL], F32)
                nc.vector.tensor_copy(out=m_f, in_=m_i32)
                m_bf = bpool.tile([P, NCOL], BF16)
                nc.scalar.activation(
                    out=m_bf, in_=m_f, func=mybir.ActivationFunctionType.Copy
                )

                s_all = bpool.tile([P, NCOL], F32)
                w_all = bpool.tile([P, NCOL], BF16)  # masked exp weights
                acc = pacc.tile([1, D], F32)
                z = pz.tile([1, 1], F32)

                for c in range(NCHUNK):
                    xt = xpool.tile([P, CHUNK, D], BF16)
                    # dtype-casting DMA (fp32 -> bf16) must use SWDGE
                    nc.gpsimd.dma_start(
                        out=xt, in_=xv[b, :, c * CHUNK : (c + 1) * CHUNK, :]
                    )
                    for g0 in range(0, CHUNK, G):
                        for j in range(g0, g0 + G):
                            n = c * CHUNK + j
                            prod = spool.tile([P, D], BF16)
                            # s_all[:, n] = sum_d x[:, n, d] * q_bf[d]
                            # route whole exp-groups the same way (per-tile
                            # alternation measurably inflates the STT cost)
                            if (n // G if GROUPROUTE else n) % SPLITMOD < NSPLIT:
                                # 2x-mode multiply on DVE, row-sum on ScalarE
                                nc.vector.tensor_tensor(
                                    out=prod,
                                    in0=xt[:, j, :],
                                    in1=q_bf,
                                    op=mybir.AluOpType.mult,
                                )
                                nc.scalar.activation(
                                    out=prod,
                                    in_=prod,
                                    func=mybir.ActivationFunctionType.Copy,
                                    accum_out=s_all[:, n : n + 1],
                                )
                            else:
                                nc.vector.scalar_tensor_tensor(
                                    out=prod,
                                    in0=xt[:, j, :],
                                    scalar=1.0,
                                    in1=q_bf,
                                    op0=mybir.AluOpType.mult,
                                    op1=mybir.AluOpType.mult,
                                    accum_out=s_all[:, n : n + 1],
                                )
                        gs = slice(c * CHUNK + g0, c * CHUNK + g0 + G)
                        e_raw = epool.tile([P, G], BF16)
                        nc.scalar.activation(
                            out=e_raw,
                            in_=s_all[:, gs],
                            func=mybir.ActivationFunctionType.Exp,
                        )
                        nc.vector.tensor_tensor(
                            out=w_all[:, gs],
                            in0=e_raw,
                            in1=m_bf[:, gs],
                            op=mybir.AluOpType.mult,
                        )
                        for j in range(g0, g0 + G):
                            n = c * CHUNK + j
                            nc.tensor.matmul(
                                acc,
                                lhsT=w_all[:, n : n + 1],
                                rhs=xt[:, j, :],
                                start=(n == 0),
                                stop=(n == NCOL - 1),
                            )

                # Z = sum over all t of the masked weights
                colsum = bpool.tile([P, 1], F32)
                nc.vector.reduce_sum(colsum, w_all, axis=mybir.AxisListType.X)
                nc.tensor.matmul(z, lhsT=colsum, rhs=ones_col, start=True, stop=True)

                zrec = epool.tile([1, 1], F32)
                nc.vector.reciprocal(zrec, z)
                out_row = epool.tile([1, D], F32)
                nc.vector.tensor_scalar_mul(out=out_row, in0=acc, scalar1=zrec)
                nc.sync.dma_start(out=out[b : b + 1, :], in_=out_row)

    _split_multi_waits(nc)
    return nc


def _run(x, mask, pool_query, trace=False):
    x = np.ascontiguousarray(np.asarray(x, dtype=np.float32))
    mask = np.ascontiguousarray(np.asarray(mask, dtype=np.int32))
    pool_query = np.ascontiguousarray(np.asarray(pool_query, dtype=np.float32))
    assert x.shape == (B, T, D) and mask.shape == (B, T)

    nc = _build_bass()
    in_maps = []
    for c in range(N_CORES):
        lo, hi = c * BC, (c + 1) * BC
        in_maps.append(
            {
                "x": np.ascontiguousarray(x[lo:hi]),
                "mask": np.ascontiguousarray(mask[lo:hi]),
                "pool_query": pool_query,
            }
        )
    res = run_bass_kernel_spmd(
        nc, in_maps, core_ids=list(range(N_CORES)), trace=trace
    )
    out = np.concatenate([r["out"] for r in res.results], axis=0)
    return out, res


def kernel(x, mask, pool_query):
    out, _ = _run(x, mask, pool_query)
    return out


# revision 15
# speedup vs baseline: 1.1320x; 1.1320x over previous
"""Attention pooling kernel for Trainium2 (Bass/Tile), SPMD over 8 NeuronCores.

Reference computation (per batch b):
    scores[t] = x[b,t,:] @ q / sqrt(D) + (1-mask[b,t]) * (-1e9)
    attn      = softmax(scores)            # over t
    out[b,:]  = sum_t attn[t] * x[b,t,:]

Strategy: data-parallel over batch (4 batches per core). One pass over x
(read once from HBM, ~64 MiB/core fp32 -> the kernel is HBM-bound at
~358 GB/s/core, floor ~187 us). To hide ALL compute under the DMA stream,
everything downstream of the load runs in bf16:

  - x[b] viewed as [128 partitions, 64 cols, 512] with t = p*64 + n,
    streamed in [128, CHUNK, 512] chunks, cast fp32->bf16 in the DMA
    (SWDGE). 32 KiB contiguous per partition per chunk at CHUNK=16.
  - scores: DVE scalar_tensor_tensor (x * q_scaled, accum over d). The
    two-tensor STT flavor has no accelerated uops (1x: ~(512+151)/0.96
    = 691 ns per [128,512] tile) - this engine is the kernel's pole at
    ~94% busy, everything else hides under it.
  - exp on ScalarE in groups of G tiles; masking is a bf16 multiply by
    m (in {0,1}) AFTER exp (exact - replaces the -1e9 bias add). Scores
    are O(0.1) so no max-subtraction is needed.
  - pooled accumulation on PE (bf16): psum[1,512] += w_col.T @ x_tile,
    over all 64 tiles of the batch, grouped with the exp stage so the PE
    gets work every ~2 us.
  - Z = sum(w) via column reduce + ones-matmul. out = acc / Z (fp32).

Measured on HW: 217 us (vs 323 us fp32r baseline). A GpSimd/ScalarE
score offload and a per-column exp-with-bias variant were both tried and
measured SLOWER (238 us): the Pool TT is ~1.4 us/tile and the offloaded
columns stall the in-order PE matmul chain; keep the DVE-only stream.
"""

import os

import numpy as np

import bass_rust as _br
import concourse.bass as bass
import concourse.tile as tile
from concourse import mybir
from concourse.bass_utils import run_bass_kernel_spmd

B, T, D = 32, 8192, 512
N_CORES = 8
BC = B // N_CORES  # batches per core
P = 128  # SBUF partitions
NCOL = T // P  # 64 tiles (columns) per batch
CHUNK = int(os.environ.get("AP_CHUNK", "16"))  # tiles per DMA chunk
NCHUNK = NCOL // CHUNK
G = int(os.environ.get("AP_G", "4"))  # tiles per exp/matmul group
SCALE = 1.0 / float(np.sqrt(np.float32(D)))
XBUFS = int(os.environ.get("AP_XBUFS", "4"))
# Of every SPLITMOD score tiles, NSPLIT take the [DVE 2x-mode multiply +
# ScalarE accumulate-copy] route; the rest use the fused (1x) DVE STT.
# Balances the DVE (the pole) against ScalarE slack. Measured on HW:
# 0/3 -> 217 us, 1/3 -> 214 us, 2/5 (+XBUFS=4) -> 188 us, 2/3 -> 235 us.
NSPLIT = int(os.environ.get("AP_NSPLIT", "2"))
SPLITMOD = int(os.environ.get("AP_SPLITMOD", "5"))
# 1 = route whole exp-groups down one path. Measured WORSE (245 us vs 205):
# consecutive slow-route columns stall the in-order PE matmul chain.
GROUPROUTE = int(os.environ.get("AP_GROUPROUTE", "0"))

F32 = mybir.dt.float32
BF16 = mybir.dt.bfloat16
I32 = mybir.dt.int32


def _split_multi_waits(nc):
    """The walrus build in this container accepts only one sync-wait command
    per instruction; hoist extra waits onto standalone EventSemaphore
    instructions placed just before (same engine, program order preserved)."""
    for f in nc.m.functions:
        for b in f.blocks:
            insts = b.instructions
            new = []
            changed = False
            for inst in insts:
                si = inst.sync_info
                if si is not None and len(si.on_wait) > 1:
                    waits = list(si.on_wait)
                    for w in waits[:-1]:
                        ies = mybir.InstEventSemaphore(
                            name=f"I-waitsplit-{nc.next_id()}", ins=[], outs=[]
                        )
                        ies.engine = inst.engine
                        ies.sync_info = _br.SyncInfo(on_wait=[w], on_update=[])
                        new.append(ies)
                    inst.sync_info = _br.SyncInfo(
                        on_wait=[waits[-1]], on_update=list(si.on_update)
                    )
                    changed = True
                new.append(inst)
            if changed:
                b.instructions = new


def _build_bass():
    nc = bass.Bass(
        "TRN2", target_bir_lowering=False, debug=False, num_devices=N_CORES
    )
    x = nc.dram_tensor("x", [BC, T, D], F32, kind="ExternalInput").ap()
    mask = nc.dram_tensor("mask", [BC, T], I32, kind="ExternalInput").ap()
    q = nc.dram_tensor("pool_query", [1, 1, D], F32, kind="ExternalInput").ap()
    out = nc.dram_tensor("out", [BC, D], F32, kind="ExternalOutput").ap()

    # t = p * NCOL + n  (partition-major): per-partition rows are contiguous
    # in DRAM, so a [128, CHUNK, 512] chunk is CHUNK*2 KiB contiguous per
    # partition.
    xv = x.rearrange("b (p n) d -> b p n d", p=P)
    mv = mask.rearrange("b (p n) -> b p n", p=P)

    with tile.TileContext(nc) as tc:
        with (
            tc.tile_pool(name="const", bufs=1) as const_pool,
            tc.tile_pool(name="xp", bufs=XBUFS) as xpool,
            tc.tile_pool(name="sp", bufs=4) as spool,
            tc.tile_pool(name="bp", bufs=2) as bpool,
            tc.tile_pool(name="ep", bufs=2) as epool,
            tc.tile_pool(name="pacc", bufs=2, space="PSUM") as pacc,
            tc.tile_pool(name="pz", bufs=2, space="PSUM") as pz,
        ):
            # q broadcast to all 128 partitions (one-time, 256 KB)
            q_bcast = const_pool.tile([P, D], F32)
            q_src = bass.AP(tensor=q.tensor, offset=q.offset, ap=[[0, P], [1, D]])
            nc.gpsimd.dma_start(out=q_bcast, in_=q_src)

            ones_col = const_pool.tile([P, 1], F32)
            nc.vector.memset(ones_col, 1.0)

            # pre-scaled bf16 query (folds the 1/sqrt(D) into q; done on
            # ScalarE so the DVE never enters a 2-port mode that would lock
            # GpSimd out of the SWDGE descriptor rings)
            q_bf = const_pool.tile([P, D], BF16)
            nc.scalar.activation(
                out=q_bf,
                in_=q_bcast,
                func=mybir.ActivationFunctionType.Copy,
                scale=float(SCALE),
            )

            for b in range(BC):
                # mask -> bf16 multiplier (1 valid, 0 pad), applied post-exp
                m_i32 = bpool.tile([P, NCOL], I32)
                nc.sync.dma_start(out=m_i32, in_=mv[b])
                m_f = bpool.tile([P, NCOL], F32)
                nc.vector.tensor_copy(out=m_f, in_=m_i32)
                m_bf = bpool.tile([P, NCOL], BF16)
                nc.scalar.activation(
                    out=m_bf, in_=m_f, func=mybir.ActivationFunctionType.Copy
                )

                s_all = bpool.tile([P, NCOL], F32)
                w_all = bpool.tile([P, NCOL], BF16)  # masked exp weights
                acc = pacc.tile([1, D], F32)
                z = pz.tile([1, 1], F32)

                for c in range(NCHUNK):
                    xt = xpool.tile([P, CHUNK, D], BF16)
                    # dtype-casting DMA (fp32 -> bf16) must use SWDGE
                    nc.gpsimd.dma_start(
                        out=xt, in_=xv[b, :, c * CHUNK : (c + 1) * CHUNK, :]
                    )
                    for g0 in range(0, CHUNK, G):
                        for j in range(g0, g0 + G):
                            n = c * CHUNK + j
                            prod = spool.tile([P, D], BF16)
                            # s_all[:, n] = sum_d x[:, n, d] * q_bf[d]
                            # route whole exp-groups the same way (per-tile
                            # alternation measurably inflates the STT cost)
                            if (n // G if GROUPROUTE else n) % SPLITMOD < NSPLIT:
                                # 2x-mode multiply on DVE, row-sum on ScalarE
                                nc.vector.tensor_tensor(
                                    out=prod,
                                    in0=xt[:, j, :],
                                    in1=q_bf,
                                    op=mybir.AluOpType.mult,
                                )
                                nc.scalar.activation(
                                    out=prod,
                                    in_=prod,
                                    func=mybir.ActivationFunctionType.Copy,
                                    accum_out=s_all[:, n : n + 1],
                                )
                            else:
                                nc.vector.scalar_tensor_tensor(
                                    out=prod,
                                    in0=xt[:, j, :],
                                    scalar=1.0,
                                    in1=q_bf,
                                    op0=mybir.AluOpType.mult,
                                    op1=mybir.AluOpType.mult,
                                    accum_out=s_all[:, n : n + 1],
                                )
                        gs = slice(c * CHUNK + g0, c * CHUNK + g0 + G)
                        e_raw = epool.tile([P, G], BF16)
                        nc.scalar.activation(
                            out=e_raw,
                            in_=s_all[:, gs],
                            func=mybir.ActivationFunctionType.Exp,
                        )
                        nc.vector.tensor_tensor(
                            out=w_all[:, gs],
                            in0=e_raw,
                            in1=m_bf[:, gs],
                            op=mybir.AluOpType.mult,
                        )
                        for j in range(g0, g0 + G):
                            n = c * CHUNK + j
                            nc.tensor.matmul(
                                acc,
                                lhsT=w_all[:, n : n + 1],
                                rhs=xt[:, j, :],
                                start=(n == 0),
                                stop=(n == NCOL - 1),
                            )

                # Z = sum over all t of the masked weights
                colsum = bpool.tile([P, 1], F32)
                nc.vector.reduce_sum(colsum, w_all, axis=mybir.AxisListType.X)
                nc.tensor.matmul(z, lhsT=colsum, rhs=ones_col, start=True, stop=True)

                zrec = epool.tile([1, 1], F32)
                nc.vector.reciprocal(zrec, z)
                out_row = epool.tile([1, D], F32)
                nc.vector.tensor_scalar_mul(out=out_row, in0=acc, scalar1=zrec)
                nc.sync.dma_start(out=out[b : b + 1, :], in_=out_row)

    _split_multi_waits(nc)
    return nc


def _run(x, mask, pool_query, trace=False):
    x = np.ascontiguousarray(np.asarray(x, dtype=np.float32))
    mask = np.ascontiguousarray(np.asarray(mask, dtype=np.int32))
    pool_query = np.ascontiguousarray(np.asarray(pool_query, dtype=np.float32))
    assert x.shape == (B, T, D) and mask.shape == (B, T)

    nc = _build_bass()
    in_maps = []
    for c in range(N_CORES):
        lo, hi = c * BC, (c + 1) * BC
        in_maps.append(
            {
                "x": np.ascontiguousarray(x[lo:hi]),
                "mask": np.ascontiguousarray(mask[lo:hi]),
                "pool_query": pool_query,
            }
        )
    res = run_bass_kernel_spmd(
        nc, in_maps, core_ids=list(range(N_CORES)), trace=trace
    )
    out = np.concatenate([r["out"] for r in res.results], axis=0)
    return out, res


def kernel(x, mask, pool_query):
    out, _ = _run(x, mask, pool_query)
    return out
